# revision 7
# baseline (speedup 1.0000x reference)
"""Trainium2 Bass kernel for nn_CrossmotionModule (gnn_message_passing).

Reference computation (B=4, M=256, T=64, Dm=512, E=768):
    rel[b,m,t,n,k] = (c[b,m,t,k] - c[b,n,t,k]) * vis[b,m,t] * vis[b,n,t]
    fea[b,t,m,(n,k)] = rel                  # (B,T,M,512)
    h   = fea @ W1 + b1                     # (B,T,M,512)
    out = [h, pos] @ W2 + b2                # (B,T,M,768)

Algebraic collapse: with p = vis (B,T,M), u0 = p*c0, u1 = p*c1, the output is
a rank-3 outer product per (b,t) plus a constant:
    out[bt,m,e] = u0[m]*G0[e] + u1[m]*G1[e] - p[m]*G2[e] + const[m,e]
where, with the host-folded fused weight V2 = W1 @ W2[:512] (512, 768):
    G0[e] = sum_n p[n] V2[2n, e]
    G1[e] = sum_n p[n] V2[2n+1, e]
    G2[e] = sum_nk (p*c)[nk] V2[nk, e]
    const = b1 @ W2[:512] + pos @ W2[512:] + b2

All matmuls run single-pass bf16 with exact split compensation
(x = xh + xl, both bf16; dropped xl*yl term is ~2^-16 relative), so the
result matches fp32 to ~1e-5 while avoiding the 2-pass fp32 PE mode.

Sharding: data-parallel over bt = (b,t) flattened; 256 rows / 8 cores = 32
rows per core. Weights replicated. No cross-device communication.
"""

import ml_dtypes
import numpy as np

B, M, T = 4, 256, 64
D_MOT, D_ABS, D_OUT = 512, 512, 768
N_CORES = 8
BT = B * T            # 256
R = BT // N_CORES     # 32 bt rows per core
E = D_OUT
RT = 4                # bt rows per output tile/DMA

BF16 = ml_dtypes.bfloat16

_CACHED_NC = None


def _split_bf16(x):
    xh = x.astype(BF16)
    xl = (x - xh.astype(np.float32)).astype(BF16)
    return xh, xl


def _build_nc():
    """Build the SPMD Bass program (identical for all 8 cores)."""
    import concourse.bacc as bacc
    import concourse.bass as bass
    import concourse.mybir as mybir
    import concourse.tile as tile

    f32 = mybir.dt.float32
    bf16 = mybir.dt.bfloat16
    PSUM = bass.MemorySpace.PSUM

    nc = bacc.Bacc("TRN2", target_bir_lowering=False, debug=False)

    # Per-core inputs (host-prepared layouts; see _prep_inputs).
    la_d = nc.dram_tensor("la", [128, 4 * 96], bf16, kind="ExternalInput")
    lb_d = nc.dram_tensor("lb", [128, 4 * 96], bf16, kind="ExternalInput")
    vh_d = nc.dram_tensor("vh", [128, 4 * E], bf16, kind="ExternalInput")
    vl_d = nc.dram_tensor("vl", [128, 4 * E], bf16, kind="ExternalInput")
    ut9_d = nc.dram_tensor("ut9", [9, R * 256], bf16, kind="ExternalInput")
    cst_d = nc.dram_tensor("cst", [128, 1536], f32, kind="ExternalInput")
    out_d = nc.dram_tensor("out", [R, M, E], f32, kind="ExternalOutput")
    # DRAM bounce buffer for the G partition reshuffle (96, 2E) -> (9, R*E).
    gscr_d = nc.dram_tensor("gscr", [3 * R, 2 * E], bf16)

    with tile.TileContext(nc) as tc:
        with tc.tile_pool(name="persist", bufs=1) as pers:
            ut9_sb = pers.tile([9, R * 256], bf16)
            g9_sb = pers.tile([9, R * E], bf16)
            cst_sb = pers.tile([128, 1536], f32)

            # ---- prologue: G[(j,r), e] via the fused weight V2 ----
            with (
                tc.tile_pool(name="pro", bufs=1) as pro,
                tc.tile_pool(name="prop", bufs=1, space=PSUM) as prop,
            ):
                la_sb = pro.tile([128, 4 * 96], bf16)
                lb_sb = pro.tile([128, 4 * 96], bf16)
                vh_sb = pro.tile([128, 4 * E], bf16)
                vl_sb = pro.tile([128, 4 * E], bf16)
                nc.sync.dma_start(la_sb[:], la_d[:])
                nc.sync.dma_start(lb_sb[:], lb_d[:])
                nc.sync.dma_start(vh_sb[:], vh_d[:])
                nc.sync.dma_start(vl_sb[:], vl_d[:])
                nc.sync.dma_start(ut9_sb[:], ut9_d[:])
                nc.sync.dma_start(cst_sb[:], cst_d[:])

                # G = Gh + Gl packed side by side: [Gh | Gl] per row.
                ghl_sb = pro.tile([3 * R, 2 * E], bf16)

                # 12 accumulation steps x 2 PSUM-bank segments:
                #   kk 0-3: lhsT = L chunks,  rhs = V2h chunks
                #   kk 4-7: lhsT = L chunks,  rhs = V2l chunks
                #   kk 8-11: lhsT = LB chunks ([0|0|Ql]), rhs = V2h chunks
                g_ps = prop.tile([3 * R, E], f32)
                for kk in range(12):
                    lsrc = lb_sb if kk >= 8 else la_sb
                    vsrc = vl_sb if 4 <= kk < 8 else vh_sb
                    kc = kk % 4
                    for lo, hi in ((0, 512), (512, 768)):
                        nc.tensor.matmul(
                            g_ps[:, lo:hi],
                            lsrc[:, kc * 96 : (kc + 1) * 96],
                            vsrc[:, kc * E + lo : kc * E + hi],
                            start=(kk == 0),
                            stop=(kk == 11),
                        )
                # Split G into exact bf16 halves: G = Gh + Gl (+ ~2^-16).
                nc.vector.tensor_copy(ghl_sb[:, 0:E], g_ps[:])
                nc.vector.tensor_sub(ghl_sb[:, E : 2 * E], g_ps[:], ghl_sb[:, 0:E])
                # Reshuffle rows (j*R+r, [h|l] e) -> partitions [Gh;Gl;Gh],
                # free (r, e), bounced through DRAM (free-form APs there).
                nc.sync.dma_start(gscr_d[:], ghl_sb[:])
                gh_v = gscr_d[:, 0:E].rearrange("(j r) e -> j r e", j=3)
                gl_v = gscr_d[:, E : 2 * E].rearrange("(j r) e -> j r e", j=3)
                nc.sync.dma_start(g9_sb[0:3].rearrange("j (r e) -> j r e", r=R), gh_v)
                nc.sync.dma_start(g9_sb[3:6].rearrange("j (r e) -> j r e", r=R), gl_v)
                nc.sync.dma_start(g9_sb[6:9].rearrange("j (r e) -> j r e", r=R), gh_v)

            # ---- main loop: out[r, m, e] = U9_r^T G9_r + const ----
            with (
                tc.tile_pool(name="mp", bufs=2, space=PSUM) as mp,
                tc.tile_pool(name="op", bufs=3) as op,
            ):
                for tp in range(R // RT):
                    out_sb = op.tile([128, RT * 1536], f32)
                    for q in range(RT):
                        r = RT * tp + q
                        ps = mp.tile([128, 1536], f32)
                        u0 = ut9_sb[:, r * 256 : r * 256 + 128]
                        u1 = ut9_sb[:, r * 256 + 128 : r * 256 + 256]
                        g0 = r * E
                        nc.tensor.matmul(ps[:, 0:512], u0, g9_sb[:, g0 : g0 + 512])
                        nc.tensor.matmul(ps[:, 512:768], u0, g9_sb[:, g0 + 512 : g0 + 768])
                        nc.tensor.matmul(ps[:, 768:1024], u1, g9_sb[:, g0 : g0 + 256])
                        nc.tensor.matmul(ps[:, 1024:1536], u1, g9_sb[:, g0 + 256 : g0 + 768])
                        nc.vector.tensor_add(
                            out_sb[:, q * 1536 : (q + 1) * 1536], ps[:], cst_sb[:]
                        )
                    nc.sync.dma_start(
                        out_d[RT * tp : RT * tp + RT].rearrange(
                            "q (p w) e -> p q w e", w=2
                        ),
                        out_sb[:].rearrange("p (q w e) -> p q w e", q=RT, w=2),
                    )
    nc.compile()
    return nc


def _prep_inputs(coords, mask, pos, w1, b1, w2, b2):
    """Host-side input sharding + weight-only constant folding."""
    nan0 = np.isnan(coords[..., 0])
    c = np.nan_to_num(coords)
    vis = np.where(nan0, np.float32(0.0), mask).astype(np.float32)

    p_all = np.ascontiguousarray(vis.transpose(0, 2, 1)).reshape(BT, M)
    c_bt = np.ascontiguousarray(c.transpose(0, 2, 1, 3)).reshape(BT, M, 2)
    q_all = (p_all[:, :, None] * c_bt).reshape(BT, 2 * M).astype(np.float32)

    W2t = w2[:D_MOT]
    W2b = w2[D_MOT:]
    const = (b1 @ W2t + b2)[None, :] + pos @ W2b          # (M, 768)
    cst_dev = np.ascontiguousarray(
        const.astype(np.float32).reshape(128, 2, D_OUT)
    ).reshape(128, 1536)

    # Fused weight V2 = W1 @ W2t, split into exact bf16 halves, each
    # chunked with 128 contraction rows per chunk.
    v2 = (w1 @ W2t).astype(np.float32)                    # (512, 768)
    v2h, v2l = _split_bf16(v2)
    vh_dev = np.ascontiguousarray(
        v2h.reshape(4, 128, D_OUT).transpose(1, 0, 2)
    ).reshape(128, 4 * D_OUT)
    vl_dev = np.ascontiguousarray(
        v2l.reshape(4, 128, D_OUT).transpose(1, 0, 2)
    ).reshape(128, 4 * D_OUT)

    # U9 rows pair with G9 rows [Gh; Gl; Gh]: [Uh; Uh; Ul].
    u0 = q_all[:, 0::2]
    u1 = q_all[:, 1::2]
    U3 = np.stack([u0, u1, -p_all], axis=0)               # (3, BT, M)
    U3 = U3.reshape(3, BT, 128, 2).transpose(0, 1, 3, 2)  # m = 2p+w
    uh, ul = _split_bf16(U3)
    U9 = np.concatenate([uh, uh, ul], axis=0)             # (9, BT, 2, 128)

    qh_all, ql_all = _split_bf16(q_all)

    in_maps = []
    for i in range(N_CORES):
        rows = slice(i * R, (i + 1) * R)
        ut9_i = np.ascontiguousarray(U9[:, rows]).reshape(9, R * 256)

        # L = [P0 | P1 | Qh] (512, 96): P0[2n]=P^T[n], P0[2n+1]=0; P1 odd rows.
        pc_t = p_all[rows].T                              # (256, R)
        la = np.zeros((512, 96), np.float32)
        la[0::2, 0:32] = pc_t
        la[1::2, 32:64] = pc_t
        la = la.astype(BF16)
        la[:, 64:96] = qh_all[rows].T
        lb = np.zeros((512, 96), BF16)
        lb[:, 64:96] = ql_all[rows].T
        la_i = np.ascontiguousarray(
            la.reshape(4, 128, 96).transpose(1, 0, 2)
        ).reshape(128, 384)
        lb_i = np.ascontiguousarray(
            lb.reshape(4, 128, 96).transpose(1, 0, 2)
        ).reshape(128, 384)
        in_maps.append(
            {
                "la": la_i,
                "lb": lb_i,
                "vh": vh_dev,
                "vl": vl_dev,
                "ut9": ut9_i,
                "cst": cst_dev,
            }
        )
    return in_maps


def _run(inputs, trace=False, trace_kwargs=None):
    from concourse.bass_utils import run_bass_kernel_spmd

    global _CACHED_NC
    if _CACHED_NC is None:
        _CACHED_NC = _build_nc()
    nc = _CACHED_NC

    coords = np.asarray(inputs["point_trajs_gt_coord"], dtype=np.float32)
    mask = np.asarray(inputs["point_trajs_visibility_mask"], dtype=np.float32)
    pos = np.asarray(inputs["pos_embed"], dtype=np.float32)
    w1 = np.asarray(inputs["fc1_w"], dtype=np.float32)
    b1 = np.asarray(inputs["fc1_b"], dtype=np.float32)
    w2 = np.asarray(inputs["fc_out_w"], dtype=np.float32)
    b2 = np.asarray(inputs["fc_out_b"], dtype=np.float32)

    in_maps = _prep_inputs(coords, mask, pos, w1, b1, w2, b2)
    res = run_bass_kernel_spmd(
        nc, in_maps, list(range(N_CORES)), trace=trace, **(trace_kwargs or {})
    )
    shards = [res.results[i]["out"] for i in range(N_CORES)]
    full = np.concatenate(shards, axis=0).reshape(B, T, M, D_OUT)
    return full, res


def kernel(**inputs):
    out, _ = _run(inputs, trace=False)
    return out


# revision 10
# speedup vs baseline: 1.1072x; 1.1072x over previous
"""Trainium2 Bass kernel for nn_CrossmotionModule (gnn_message_passing).

Reference computation (B=4, M=256, T=64, Dm=512, E=768):
    rel[b,m,t,n,k] = (c[b,m,t,k] - c[b,n,t,k]) * vis[b,m,t] * vis[b,n,t]
    fea[b,t,m,(n,k)] = rel                  # (B,T,M,512)
    h   = fea @ W1 + b1                     # (B,T,M,512)
    out = [h, pos] @ W2 + b2                # (B,T,M,768)

Algebraic collapse: with p = vis (B,T,M), u0 = p*c0, u1 = p*c1, the output is
a rank-3 outer product per (b,t) plus a constant:
    out[bt,m,e] = u0[m]*G0[e] + u1[m]*G1[e] - p[m]*G2[e] + const[m,e]
where, with the host-folded fused weight V2 = W1 @ W2[:512] (512, 768):
    G0[e] = sum_n p[n] V2[2n, e]
    G1[e] = sum_n p[n] V2[2n+1, e]
    G2[e] = sum_nk (p*c)[nk] V2[nk, e]
    const = b1 @ W2[:512] + pos @ W2[512:] + b2

All matmuls run single-pass bf16 with exact split compensation
(x = xh + xl, both bf16; dropped xl*yl term is ~2^-16 relative), so the
result matches fp32 to ~1e-5 while avoiding the 2-pass fp32 PE mode.

Sharding: data-parallel over bt = (b,t) flattened; 256 rows / 8 cores = 32
rows per core. Weights replicated. No cross-device communication.
"""

import ml_dtypes
import numpy as np

B, M, T = 4, 256, 64
D_MOT, D_ABS, D_OUT = 512, 512, 768
N_CORES = 8
BT = B * T            # 256
R = BT // N_CORES     # 32 bt rows per core
E = D_OUT
RT = 2                # bt rows per output tile/DMA

BF16 = ml_dtypes.bfloat16

_CACHED_NC = None


def _split_bf16(x):
    xh = x.astype(BF16)
    xl = (x - xh.astype(np.float32)).astype(BF16)
    return xh, xl


def _build_nc():
    """Build the SPMD Bass program (identical for all 8 cores)."""
    import concourse.bacc as bacc
    import concourse.bass as bass
    import concourse.mybir as mybir
    import concourse.tile as tile

    f32 = mybir.dt.float32
    bf16 = mybir.dt.bfloat16
    PSUM = bass.MemorySpace.PSUM

    nc = bacc.Bacc("TRN2", target_bir_lowering=False, debug=False)

    # Per-core inputs (host-prepared layouts; see _prep_inputs).
    la_d = nc.dram_tensor("la", [128, 4 * 96], bf16, kind="ExternalInput")
    lb_d = nc.dram_tensor("lb", [128, 4 * 96], bf16, kind="ExternalInput")
    vh_d = nc.dram_tensor("vh", [128, 4 * E], bf16, kind="ExternalInput")
    vl_d = nc.dram_tensor("vl", [128, 4 * E], bf16, kind="ExternalInput")
    ut9_d = nc.dram_tensor("ut9", [9, R * 256], bf16, kind="ExternalInput")
    cst_d = nc.dram_tensor("cst", [128, 1536], f32, kind="ExternalInput")
    out_d = nc.dram_tensor("out", [R, M, E], f32, kind="ExternalOutput")
    # DRAM bounce buffer for the G partition reshuffle, laid out so the
    # read back into (9, R*E) SBUF partitions is one plain fat DMA.
    gscr_d = nc.dram_tensor("gscr", [9, R * E], bf16)

    with tile.TileContext(nc) as tc:
        with tc.tile_pool(name="persist", bufs=1) as pers:
            ut9_sb = pers.tile([9, R * 256], bf16)
            g9_sb = pers.tile([9, R * E], bf16)
            cst_sb = pers.tile([128, 1536], f32)

            # ---- prologue: G[(j,r), e] via the fused weight V2 ----
            with (
                tc.tile_pool(name="pro", bufs=1) as pro,
                tc.tile_pool(name="prop", bufs=1, space=PSUM) as prop,
            ):
                la_sb = pro.tile([128, 4 * 96], bf16)
                lb_sb = pro.tile([128, 4 * 96], bf16)
                vh_sb = pro.tile([128, 4 * E], bf16)
                vl_sb = pro.tile([128, 4 * E], bf16)
                nc.sync.dma_start(la_sb[:], la_d[:])
                nc.sync.dma_start(lb_sb[:], lb_d[:])
                nc.sync.dma_start(vh_sb[:], vh_d[:])
                nc.sync.dma_start(vl_sb[:], vl_d[:])
                nc.sync.dma_start(ut9_sb[:], ut9_d[:])
                nc.sync.dma_start(cst_sb[:], cst_d[:])

                # G = Gh + Gl packed side by side: [Gh | Gl] per row.
                ghl_sb = pro.tile([3 * R, 2 * E], bf16)

                # 12 accumulation steps x 2 PSUM-bank segments:
                #   kk 0-3: lhsT = L chunks,  rhs = V2h chunks
                #   kk 4-7: lhsT = L chunks,  rhs = V2l chunks
                #   kk 8-11: lhsT = LB chunks ([0|0|Ql]), rhs = V2h chunks
                g_ps = prop.tile([3 * R, E], f32)
                for kk in range(12):
                    lsrc = lb_sb if kk >= 8 else la_sb
                    vsrc = vl_sb if 4 <= kk < 8 else vh_sb
                    kc = kk % 4
                    for lo, hi in ((0, 512), (512, 768)):
                        nc.tensor.matmul(
                            g_ps[:, lo:hi],
                            lsrc[:, kc * 96 : (kc + 1) * 96],
                            vsrc[:, kc * E + lo : kc * E + hi],
                            start=(kk == 0),
                            stop=(kk == 11),
                        )
                # Split G into exact bf16 halves: G = Gh + Gl (+ ~2^-16).
                nc.vector.tensor_copy(ghl_sb[:, 0:E], g_ps[:])
                nc.vector.tensor_sub(ghl_sb[:, E : 2 * E], g_ps[:], ghl_sb[:, 0:E])
                # Reshuffle rows (j*R+r, [h|l] e) -> [Gh;Gl;Gh] x (r, e).
                # The scatter happens on the DRAM WRITE side (src keeps 96
                # partitions -> full SDMA parallelism); the read back is one
                # plain (9, R*E) DMA with 48KB-per-partition descriptors.
                nc.sync.dma_start(
                    gscr_d[0:3].rearrange("j (r e) -> (j r) e", r=R),
                    ghl_sb[:, 0:E],
                )
                nc.sync.dma_start(
                    gscr_d[3:6].rearrange("j (r e) -> (j r) e", r=R),
                    ghl_sb[:, E : 2 * E],
                )
                nc.sync.dma_start(
                    gscr_d[6:9].rearrange("j (r e) -> (j r) e", r=R),
                    ghl_sb[:, 0:E],
                )
                nc.sync.dma_start(g9_sb[:], gscr_d[:])

            # ---- main loop: out[r, m, e] = U9_r^T G9_r + const ----
            with (
                tc.tile_pool(name="mp", bufs=2, space=PSUM) as mp,
                tc.tile_pool(name="op", bufs=3) as op,
            ):
                for tp in range(R // RT):
                    out_sb = op.tile([128, RT * 1536], f32)
                    for q in range(RT):
                        r = RT * tp + q
                        ps = mp.tile([128, 1536], f32)
                        u0 = ut9_sb[:, r * 256 : r * 256 + 128]
                        u1 = ut9_sb[:, r * 256 + 128 : r * 256 + 256]
                        g0 = r * E
                        nc.tensor.matmul(ps[:, 0:512], u0, g9_sb[:, g0 : g0 + 512])
                        nc.tensor.matmul(ps[:, 512:768], u0, g9_sb[:, g0 + 512 : g0 + 768])
                        nc.tensor.matmul(ps[:, 768:1024], u1, g9_sb[:, g0 : g0 + 256])
                        nc.tensor.matmul(ps[:, 1024:1536], u1, g9_sb[:, g0 + 256 : g0 + 768])
                        nc.vector.tensor_add(
                            out_sb[:, q * 1536 : (q + 1) * 1536], ps[:], cst_sb[:]
                        )
                    nc.sync.dma_start(
                        out_d[RT * tp : RT * tp + RT].rearrange(
                            "q (p w) e -> p q w e", w=2
                        ),
                        out_sb[:].rearrange("p (q w e) -> p q w e", q=RT, w=2),
                    )
    nc.compile()
    return nc


def _prep_inputs(coords, mask, pos, w1, b1, w2, b2):
    """Host-side input sharding + weight-only constant folding."""
    nan0 = np.isnan(coords[..., 0])
    c = np.nan_to_num(coords)
    vis = np.where(nan0, np.float32(0.0), mask).astype(np.float32)

    p_all = np.ascontiguousarray(vis.transpose(0, 2, 1)).reshape(BT, M)
    c_bt = np.ascontiguousarray(c.transpose(0, 2, 1, 3)).reshape(BT, M, 2)
    q_all = (p_all[:, :, None] * c_bt).reshape(BT, 2 * M).astype(np.float32)

    W2t = w2[:D_MOT]
    W2b = w2[D_MOT:]
    const = (b1 @ W2t + b2)[None, :] + pos @ W2b          # (M, 768)
    cst_dev = np.ascontiguousarray(
        const.astype(np.float32).reshape(128, 2, D_OUT)
    ).reshape(128, 1536)

    # Fused weight V2 = W1 @ W2t, split into exact bf16 halves, each
    # chunked with 128 contraction rows per chunk.
    v2 = (w1 @ W2t).astype(np.float32)                    # (512, 768)
    v2h, v2l = _split_bf16(v2)
    vh_dev = np.ascontiguousarray(
        v2h.reshape(4, 128, D_OUT).transpose(1, 0, 2)
    ).reshape(128, 4 * D_OUT)
    vl_dev = np.ascontiguousarray(
        v2l.reshape(4, 128, D_OUT).transpose(1, 0, 2)
    ).reshape(128, 4 * D_OUT)

    # U9 rows pair with G9 rows [Gh; Gl; Gh]: [Uh; Uh; Ul].
    u0 = q_all[:, 0::2]
    u1 = q_all[:, 1::2]
    U3 = np.stack([u0, u1, -p_all], axis=0)               # (3, BT, M)
    U3 = U3.reshape(3, BT, 128, 2).transpose(0, 1, 3, 2)  # m = 2p+w
    uh, ul = _split_bf16(U3)
    U9 = np.concatenate([uh, uh, ul], axis=0)             # (9, BT, 2, 128)

    qh_all, ql_all = _split_bf16(q_all)

    in_maps = []
    for i in range(N_CORES):
        rows = slice(i * R, (i + 1) * R)
        ut9_i = np.ascontiguousarray(U9[:, rows]).reshape(9, R * 256)

        # L = [P0 | P1 | Qh] (512, 96): P0[2n]=P^T[n], P0[2n+1]=0; P1 odd rows.
        pc_t = p_all[rows].T                              # (256, R)
        la = np.zeros((512, 96), np.float32)
        la[0::2, 0:32] = pc_t
        la[1::2, 32:64] = pc_t
        la = la.astype(BF16)
        la[:, 64:96] = qh_all[rows].T
        lb = np.zeros((512, 96), BF16)
        lb[:, 64:96] = ql_all[rows].T
        la_i = np.ascontiguousarray(
            la.reshape(4, 128, 96).transpose(1, 0, 2)
        ).reshape(128, 384)
        lb_i = np.ascontiguousarray(
            lb.reshape(4, 128, 96).transpose(1, 0, 2)
        ).reshape(128, 384)
        in_maps.append(
            {
                "la": la_i,
                "lb": lb_i,
                "vh": vh_dev,
                "vl": vl_dev,
                "ut9": ut9_i,
                "cst": cst_dev,
            }
        )
    return in_maps


def _run(inputs, trace=False, trace_kwargs=None):
    from concourse.bass_utils import run_bass_kernel_spmd

    global _CACHED_NC
    if _CACHED_NC is None:
        _CACHED_NC = _build_nc()
    nc = _CACHED_NC

    coords = np.asarray(inputs["point_trajs_gt_coord"], dtype=np.float32)
    mask = np.asarray(inputs["point_trajs_visibility_mask"], dtype=np.float32)
    pos = np.asarray(inputs["pos_embed"], dtype=np.float32)
    w1 = np.asarray(inputs["fc1_w"], dtype=np.float32)
    b1 = np.asarray(inputs["fc1_b"], dtype=np.float32)
    w2 = np.asarray(inputs["fc_out_w"], dtype=np.float32)
    b2 = np.asarray(inputs["fc_out_b"], dtype=np.float32)

    in_maps = _prep_inputs(coords, mask, pos, w1, b1, w2, b2)
    res = run_bass_kernel_spmd(
        nc, in_maps, list(range(N_CORES)), trace=trace, **(trace_kwargs or {})
    )
    shards = [res.results[i]["out"] for i in range(N_CORES)]
    full = np.concatenate(shards, axis=0).reshape(B, T, M, D_OUT)
    return full, res


def kernel(**inputs):
    out, _ = _run(inputs, trace=False)
    return out
